# revision 1
# baseline (speedup 1.0000x reference)
"""Trainium2 Bass kernel for nn_BERT_KNNCL_35527969473209 (retrieval_knn).

Contract: kernel(**inputs) takes the FULL inputs (liner_q [128,768] f32,
feature_queue [65536,768] f32, label_q [128] int, label_queue [65536] int)
and returns the FULL output [640, 64513] f32, matching:

    q = l2norm(liner_q); cos = q @ feature_queue.T
    pos = top_k(cos, 5) -> [640,1]
    neg = sort_desc(where(label match, -inf, cos))[:, :64512], rows repeated 5x
    out = concat([pos, neg], -1) / 0.07

Strategy (SPMD over 8 NeuronCores, queue-dim sharded):
  host: l2norm+1/T fold into q^T; transpose per-core feature chunk;
        per-core penalty matrix (-1e38 at label matches).
  core c: S = q^T.T @ fqt_c  [128 x 8192] (PE, fp32)
          top8/row via DVE InstMax (pre-mask); S += pen;
          per-row descending bitonic sort of the 8192-chunk (DVE);
          AllToAll 16-row shards -> row-owner core;
          bitonic merge of 8 sorted runs (DVE + DMA relabel between
          cross-partition stages); top5 = max8 of gathered top8s;
          write the [80 x 64513] shard (5x row replication via DMA).
  host: concatenate the 8 shards.

Notes:
- Sort network: normalized bitonic (all comparators same direction,
  mirror stage per merge round) emitted as DVE tensor_tensor max/min
  pairs over 3D access patterns; mirror stages read one operand with a
  negative last-dim stride. Both SBUF inputs of a DVE op must share the
  base partition (hardware constraint NCC_IBIR297); the OUTPUT may live
  at a different base partition, which the merge phase exploits: partner
  runs are DMA-aligned into V64/Y64 scratch, max-half is written to
  partitions [0:64), min-half to [64:128).
- Validated on trn2 hardware vs the jax reference: max abs err ~7e-5 on
  output scale ~82 (fp32 matmul accumulation-order noise only).
- Cost-model (TimelineSim) device estimate: ~1.54 ms/core, DVE-bound
  (~1.25 ms of compare-exchange at fp32 1x = 1 elem/lane/cycle).
"""

import sys

import numpy as np

for _p in ("/opt/trn_rl_repo", "/root/.axon_site/_ro/trn_rl_repo"):
    if _p not in sys.path:
        sys.path.append(_p)

import concourse.bass as bass  # noqa: E402
import concourse.tile as tile  # noqa: E402
from concourse import bacc, mybir  # noqa: E402
from concourse.bass_utils import run_bass_kernel_spmd  # noqa: E402

F32 = mybir.dt.float32
MAX = mybir.AluOpType.max
MIN = mybir.AluOpType.min
ADD = mybir.AluOpType.add

NCORES = 8
B = 128
NROW = B // NCORES
NLBL = 64
TOPK = 5
KC = 8192
H = 768
T_TEMP = 0.07
# Emit phase-4 rounds 1..log2(KC)-1 separately per column half so the left
# half's sort can overlap the right half's matmul/DMA stream.
SPLIT_PHASE4 = True


def _log2i(n):
    k = n.bit_length() - 1
    assert (1 << k) == n
    return k


def build_nc(KC=KC, H=H):
    K = KC * NCORES
    POS = K // NLBL
    LAST = KC - POS
    NEG = K - POS
    OUTC = NEG + 1
    HC = H // 128
    JC = KC // 512

    nc = bacc.Bacc("TRN2", target_bir_lowering=False, debug=False,
                   num_devices=NCORES)

    qT = nc.dram_tensor("qT", [H, B], F32, kind="ExternalInput")
    fqt = nc.dram_tensor("fqt", [H, KC], F32, kind="ExternalInput")
    pen = nc.dram_tensor("pen", [B, KC], F32, kind="ExternalInput")
    out = nc.dram_tensor("out", [NROW * TOPK, OUTC], F32, kind="ExternalOutput")

    with tile.TileContext(nc) as tc:
        with (
            tc.tile_pool(name="fq", bufs=12) as fpool,
            tc.tile_pool(name="psum", bufs=4, space="PSUM") as ppool,
            tc.tile_pool(name="dram", bufs=1, space="DRAM") as dpool,
        ):
            S = nc.alloc_sbuf_tensor("S", [128, KC], F32).ap()
            T = nc.alloc_sbuf_tensor("T", [128, KC], F32).ap()
            V64 = nc.alloc_sbuf_tensor("V64", [64, KC], F32).ap()
            Y64 = nc.alloc_sbuf_tensor("Y64", [64, KC], F32).ap()
            qt_sb = nc.alloc_sbuf_tensor("qt_sb", [128, H], F32).ap()
            top8 = nc.alloc_sbuf_tensor("top8", [128, 8], F32).ap()
            T8 = nc.alloc_sbuf_tensor("T8", [16, 64], F32).ap()
            pos8 = nc.alloc_sbuf_tensor("pos8", [16, 8], F32).ap()

            a2a_in = dpool.tile([B, KC + 8], F32, tag="a2a_in")
            a2a_out = dpool.tile([B, KC + 8], F32, tag="a2a_out")

            # ---- load q^T and penalty ----
            for hc in range(HC):
                nc.sync.dma_start(qt_sb[:, hc * 128:(hc + 1) * 128],
                                  qT[hc * 128:(hc + 1) * 128, :])
            nc.sync.dma_start(T[:], pen[:])

            # top8 candidates per 512-col block (pre-mask), masked in-loop so
            # the DVE work hides under the DMA/PE-bound matmul pipeline
            t8c = nc.alloc_sbuf_tensor("t8c", [128, 8 * JC], F32).ap()

            # ---- matmul S = q @ F^T ----
            for jc in range(JC):
                ftiles = []
                for hc in range(HC):
                    ft = fpool.tile([128, 512], F32, tag="ft")
                    nc.sync.dma_start(
                        ft[:], fqt[hc * 128:(hc + 1) * 128,
                                   jc * 512:(jc + 1) * 512])
                    ftiles.append(ft)
                ps = ppool.tile([128, 512], F32, tag="ps")
                for hc in range(HC):
                    nc.tensor.matmul(ps[:], qt_sb[:, hc * 128:(hc + 1) * 128],
                                     ftiles[hc][:], start=(hc == 0),
                                     stop=(hc == HC - 1))
                sl = slice(jc * 512, (jc + 1) * 512)
                nc.scalar.activation(S[:, sl], ps[:],
                                     mybir.ActivationFunctionType.Copy)
                nc.vector.max(t8c[:, jc * 8:(jc + 1) * 8], S[:, sl])
                nc.vector.tensor_tensor(S[:, sl], S[:, sl], T[:, sl], ADD)

            # ---- top8 per row = max over per-block candidates ----
            nc.vector.max(top8[:], t8c[:])

            # ---- per-row descending bitonic sort of the chunk ----
            cur, oth = S, T

            def halving(s, c0=0, c1=None):
                nonlocal cur, oth
                c1 = KC if c1 is None else c1
                a = cur[:, c0:c1].rearrange("p (b two s) -> p b two s",
                                            two=2, s=s)
                o = oth[:, c0:c1].rearrange("p (b two s) -> p b two s",
                                            two=2, s=s)
                nc.vector.tensor_tensor(o[:, :, 0, :], a[:, :, 0, :],
                                        a[:, :, 1, :], MAX)
                nc.vector.tensor_tensor(o[:, :, 1, :], a[:, :, 0, :],
                                        a[:, :, 1, :], MIN)
                cur, oth = oth, cur

            def sort_rounds(kmax, c0=0, c1=None):
                nonlocal cur, oth
                c1 = KC if c1 is None else c1
                for k in range(1, kmax + 1):
                    m = 1 << k
                    a = cur[:, c0:c1].rearrange("p (b m) -> p b m", m=m)
                    o = oth[:, c0:c1].rearrange("p (b m) -> p b m", m=m)
                    lo = a[:, :, 0:m // 2]
                    hi = a[:, :, m // 2:m]
                    nc.vector.tensor_tensor(o[:, :, 0:m // 2], lo,
                                            hi[:, :, ::-1], MAX)
                    nc.vector.tensor_tensor(o[:, :, m // 2:m], hi,
                                            lo[:, :, ::-1], MIN)
                    cur, oth = oth, cur
                    s = m // 4
                    while s >= 1:
                        halving(s, c0, c1)
                        s //= 2

            if SPLIT_PHASE4:
                kh = _log2i(KC) - 1
                sort_rounds(kh, 0, KC // 2)       # 78 stages (even): ends in S
                sort_rounds(kh, KC // 2, KC)      # 78 stages (even): ends in S
                # final round: mirror over full KC + halvings
                m = KC
                a = cur.rearrange("p (b m) -> p b m", m=m)
                o = oth.rearrange("p (b m) -> p b m", m=m)
                lo = a[:, :, 0:m // 2]
                hi = a[:, :, m // 2:m]
                nc.vector.tensor_tensor(o[:, :, 0:m // 2], lo,
                                        hi[:, :, ::-1], MAX)
                nc.vector.tensor_tensor(o[:, :, m // 2:m], hi,
                                        lo[:, :, ::-1], MIN)
                cur, oth = oth, cur
                s = m // 4
                while s >= 1:
                    halving(s)
                    s //= 2
            else:
                sort_rounds(_log2i(KC))

            # ---- stage for A2A ----
            nc.gpsimd.dma_start(a2a_in[:, 0:KC], cur[:])
            nc.gpsimd.dma_start(a2a_in[:, KC:KC + 8], top8[:])

            # ---- AllToAll (16-row shards) ----
            nc.gpsimd.collective_compute(
                "AllToAll", mybir.AluOpType.bypass,
                replica_groups=[list(range(NCORES))],
                ins=[a2a_in.opt()], outs=[a2a_out.opt()])

            # ---- load merge tile + top8 gather ----
            pos = {}
            for c in range(NCORES):
                g = (c // 2) if c % 2 == 0 else 4 + c // 2
                pos[c] = g
                nc.gpsimd.dma_start(cur[g * 16:(g + 1) * 16, :],
                                    a2a_out[c * 16:(c + 1) * 16, 0:KC])
                nc.gpsimd.dma_start(T8[:, c * 8:(c + 1) * 8],
                                    a2a_out[c * 16:(c + 1) * 16, KC:KC + 8])

            nc.vector.max(pos8[:], T8[:])

            # ---- merge 8 sorted runs ----
            def cross(pairs, rev, skip_v=False):
                nonlocal cur, oth
                if not skip_v:
                    for i, (lc, uc) in enumerate(pairs):
                        nc.gpsimd.dma_start(
                            V64[i * 16:(i + 1) * 16, :],
                            cur[pos[lc] * 16:(pos[lc] + 1) * 16, :])
                    vin = V64
                else:
                    vin = cur[0:64, :]
                for i, (lc, uc) in enumerate(pairs):
                    nc.gpsimd.dma_start(
                        Y64[i * 16:(i + 1) * 16, :],
                        cur[pos[uc] * 16:(pos[uc] + 1) * 16, :])
                y = Y64[:, ::-1] if rev else Y64[:]
                v = vin[:, ::-1] if rev else vin[:]
                nc.vector.tensor_tensor(oth[0:64, :], vin[:], y, MAX)
                nc.vector.tensor_tensor(oth[64:128, :], Y64[:], v, MIN)
                for i, (lc, uc) in enumerate(pairs):
                    pos[lc] = i
                    pos[uc] = 4 + i
                cur, oth = oth, cur

            def free_stages():
                s = KC // 2
                while s >= 1:
                    halving(s)
                    s //= 2

            cross([(0, 1), (2, 3), (4, 5), (6, 7)], rev=True, skip_v=True)
            free_stages()
            cross([(0, 3), (1, 2), (4, 7), (5, 6)], rev=True)
            cross([(0, 1), (2, 3), (4, 5), (6, 7)], rev=False)
            free_stages()
            cross([(0, 7), (1, 6), (2, 5), (3, 4)], rev=True)
            cross([(0, 2), (1, 3), (4, 6), (5, 7)], rev=False)
            cross([(0, 1), (2, 3), (4, 5), (6, 7)], rev=False)
            free_stages()

            # ---- outputs ----
            grp_chunk = sorted(range(8), key=lambda c: pos[c])
            fin = cur
            R2 = out.ap().flatten().rearrange("(r x) -> r x", x=TOPK * OUTC)
            for t in range(TOPK):
                for g in range(8):
                    cg = grp_chunk[g]
                    L = KC if cg < 7 else LAST
                    dst = R2[:, t * OUTC + 1 + cg * KC:
                             t * OUTC + 1 + cg * KC + L]
                    src = fin[g * 16:(g + 1) * 16, 0:L]
                    eng = [nc.gpsimd, nc.sync, nc.scalar][(t * 8 + g) % 3]
                    eng.dma_start(dst, src)
                with nc.allow_non_contiguous_dma(reason="16 scattered f32"):
                    nc.sync.dma_start(R2[:, t * OUTC:t * OUTC + 1],
                                      pos8[:, t:t + 1])

    nc.compile()
    return nc


_NC_CACHE = {}


def _get_nc():
    if "nc" not in _NC_CACHE:
        _NC_CACHE["nc"] = build_nc()
    return _NC_CACHE["nc"]


def host_inputs(liner_q, feature_queue, label_q, label_queue, KC=KC,
                T_temp=T_TEMP):
    lq = np.asarray(liner_q, dtype=np.float32)
    fq = np.asarray(feature_queue, dtype=np.float32)
    lbq = np.asarray(label_q).reshape(-1)
    lbQ = np.asarray(label_queue).reshape(-1)
    nrm = np.sqrt((lq * lq).sum(axis=1, keepdims=True))
    q = (lq / nrm / np.float32(T_temp)).astype(np.float32)
    qT = np.ascontiguousarray(q.T)
    in_maps = []
    for c in range(NCORES):
        sl = slice(c * KC, (c + 1) * KC)
        fqt_c = np.ascontiguousarray(fq[sl, :].T)
        pen_c = np.where(lbq[:, None] == lbQ[None, sl], np.float32(-1e38),
                         np.float32(0.0)).astype(np.float32)
        in_maps.append({"qT": qT, "fqt": fqt_c, "pen": pen_c})
    return in_maps


def _get_runner():
    """Cached jitted SPMD executable (avoids re-trace/re-compile per call)."""
    if "runner" in _NC_CACHE:
        return _NC_CACHE["runner"]
    import jax
    from jax.sharding import Mesh, NamedSharding, PartitionSpec
    from jax.experimental.shard_map import shard_map
    from concourse import bass2jax

    nc = _get_nc()
    partition_name = (nc.partition_id_tensor.name
                      if nc.partition_id_tensor else None)
    in_names, out_names, out_avals, out_shapes = [], [], [], []
    for alloc in nc.m.functions[0].allocations:
        if not isinstance(alloc, mybir.MemoryLocationSet):
            continue
        name = alloc.memorylocations[0].name
        if alloc.kind == "ExternalInput":
            if name != partition_name:
                in_names.append(name)
        elif alloc.kind == "ExternalOutput":
            out_names.append(name)
            shape = tuple(alloc.tensor_shape)
            dtype = mybir.dt.np(alloc.dtype)
            out_avals.append(jax.core.ShapedArray(shape, dtype))
            out_shapes.append((shape, dtype))
    n_params = len(in_names)
    all_in = list(in_names) + list(out_names)
    if partition_name is not None:
        all_in.append(partition_name)

    def _body(*args):
        operands = list(args)
        if partition_name is not None:
            operands.append(bass2jax.partition_id_tensor())
        return tuple(bass2jax._bass_exec_p.bind(
            *operands, out_avals=tuple(out_avals), in_names=tuple(all_in),
            out_names=tuple(out_names), lowering_input_output_aliases=(),
            sim_require_finite=True, sim_require_nnan=True, nc=nc))

    devices = jax.devices()[:NCORES]
    mesh = Mesh(np.asarray(devices), ("core",))
    fn = jax.jit(
        shard_map(_body, mesh=mesh,
                  in_specs=(PartitionSpec("core"),) * (n_params + len(out_names)),
                  out_specs=(PartitionSpec("core"),) * len(out_names),
                  check_rep=False),
        keep_unused=True)
    sharding = NamedSharding(mesh, PartitionSpec("core"))

    import jax.numpy as jnp
    _zeros = jax.jit(
        lambda: tuple(jnp.zeros((NCORES * s[0], *s[1:]), d)
                      for (s, d) in out_shapes),
        out_shardings=tuple(sharding for _ in out_shapes))

    def prepare(in_maps):
        per_core = [[np.asarray(m[nm]) for nm in in_names] for m in in_maps]
        concat_in = [np.concatenate([per_core[c][i] for c in range(NCORES)],
                                    axis=0) for i in range(n_params)]
        dev_in = [jax.device_put(a, sharding) for a in concat_in]
        return dev_in

    def execute(dev_in):
        return fn(*dev_in, *_zeros())

    def runner(in_maps):
        outs = execute(prepare(in_maps))
        return np.asarray(outs[0])  # [NCORES*80, OUTC], core-major

    runner.prepare = prepare
    runner.execute = execute
    _NC_CACHE["runner"] = runner
    return runner


def run(inputs, trace=False, **kw):
    """Reference-path runner (used by test.py; returns BassKernelResults)."""
    nc = _get_nc()
    in_maps = host_inputs(**inputs)
    res = run_bass_kernel_spmd(nc, in_maps, core_ids=list(range(NCORES)),
                               trace=trace, **kw)
    full = np.concatenate([r["out"] for r in res.results], axis=0)
    return full, res


def kernel(liner_q, feature_queue, label_q, label_queue):
    inputs = dict(liner_q=liner_q, feature_queue=feature_queue,
                  label_q=label_q, label_queue=label_queue)
    try:
        runner = _get_runner()
        return runner(host_inputs(**inputs))
    except Exception:
        full, _ = run(inputs)
        return full



# revision 3
# speedup vs baseline: 4.2875x; 4.2875x over previous
"""Trainium2 Bass kernel for nn_BERT_KNNCL_35527969473209 (retrieval_knn).

Contract: kernel(**inputs) takes the FULL inputs (liner_q [128,768] f32,
feature_queue [65536,768] f32, label_q [128] int, label_queue [65536] int)
and returns the FULL output [640, 64513] f32, matching:

    q = l2norm(liner_q); cos = q @ feature_queue.T
    pos = top_k(cos, 5) -> [640,1]
    neg = sort_desc(where(label match, -inf, cos))[:, :64512], rows repeated 5x
    out = concat([pos, neg], -1) / 0.07

Strategy (SPMD over 8 NeuronCores, queue-dim sharded), v2 = bf16 sort:
  host: l2norm+1/T fold into q^T (bf16); per-core feature chunk transposed
        (bf16); per-core penalty matrix (-1e38 at label matches, bf16).
  core c: S = q^T.T @ fqt_c  [128 x 8192] (PE bf16 -> PSUM f32)
          top8/row via DVE InstMax (f32, pre-mask); S = PSUM + pen (bf16);
          per-row descending bitonic sort of the 8192-chunk in bf16 using
          DVE scalar_tensor_tensor (bypass,max/min) - the InstTensorScalarPtr
          form runs at 4x (2-byte packed SBUF operands) vs 1x for the
          InstTensorTensor form at fp32;
          AllToAll 16-row shards (bf16 payload + bitcast f32 top8);
          bitonic merge of 8 sorted runs (DVE stt + DMA relabel staged
          across 3 queues); top5 = max8 of gathered top8s (f32);
          convert merged rows to f32 on the Scalar engine;
          write the [80 x 64513] shard (5x row replication via DMA).
  host: concatenate the 8 shards.

Numerics: bf16 quantization of the (already 1/T-scaled) sims bounds the
elementwise error by ~0.5 ULP(bf16) ~ 0.25 at |x|~82 -> rel-to-scale
~3e-3, well inside the 2e-2 gate; the pos column is computed from f32
top8 candidates. Sorting quantized keys is exact on the quantized values,
so no error accumulates across the 136 compare-exchange stages.
"""

import sys

import numpy as np

for _p in ("/opt/trn_rl_repo", "/root/.axon_site/_ro/trn_rl_repo"):
    if _p not in sys.path:
        sys.path.append(_p)

import concourse.bass as bass  # noqa: E402
import concourse.tile as tile  # noqa: E402
from concourse import bacc, mybir  # noqa: E402
from concourse.bass_utils import run_bass_kernel_spmd  # noqa: E402

F32 = mybir.dt.float32
BF16 = mybir.dt.bfloat16
MAX = mybir.AluOpType.max
MIN = mybir.AluOpType.min
ADD = mybir.AluOpType.add
BYP = mybir.AluOpType.bypass

NCORES = 8
B = 128
NROW = B // NCORES
NLBL = 64
TOPK = 5
KC = 8192
H = 768
T_TEMP = 0.07
# Emit phase-4 rounds 1..log2(KC)-1 separately per column half so the left
# half's sort can overlap the right half's matmul/DMA stream.
SPLIT_PHASE4 = True


def _log2i(n):
    k = n.bit_length() - 1
    assert (1 << k) == n
    return k


def build_nc(KC=KC, H=H):
    K = KC * NCORES
    POS = K // NLBL
    LAST = KC - POS
    NEG = K - POS
    OUTC = NEG + 1
    HC = H // 128
    JC = KC // 512
    PAY = KC + 16  # bf16 payload cols: sorted chunk + bitcast f32 top8

    nc = bacc.Bacc("TRN2", target_bir_lowering=False, debug=False,
                   num_devices=NCORES)

    qT = nc.dram_tensor("qT", [H, B], BF16, kind="ExternalInput")
    fqt = nc.dram_tensor("fqt", [H, KC], BF16, kind="ExternalInput")
    pen = nc.dram_tensor("pen", [B, KC], BF16, kind="ExternalInput")
    out = nc.dram_tensor("out", [NROW * TOPK, OUTC], F32, kind="ExternalOutput")

    with tile.TileContext(nc) as tc:
        with (
            tc.tile_pool(name="fq", bufs=12) as fpool,
            tc.tile_pool(name="psum", bufs=4, space="PSUM") as ppool,
            tc.tile_pool(name="dram", bufs=1, space="DRAM") as dpool,
        ):
            S = nc.alloc_sbuf_tensor("S", [128, KC], BF16).ap()
            T = nc.alloc_sbuf_tensor("T", [128, KC], BF16).ap()
            V64 = nc.alloc_sbuf_tensor("V64", [64, KC], BF16).ap()
            Y64 = nc.alloc_sbuf_tensor("Y64", [64, KC], BF16).ap()
            OUT32 = nc.alloc_sbuf_tensor("OUT32", [128, KC], F32).ap()
            qt_sb = nc.alloc_sbuf_tensor("qt_sb", [128, H], BF16).ap()
            U32 = nc.alloc_sbuf_tensor("U32", [128, 512], F32).ap()
            top8 = nc.alloc_sbuf_tensor("top8", [128, 8], F32).ap()
            T8 = nc.alloc_sbuf_tensor("T8", [16, 128], BF16).ap()
            pos8 = nc.alloc_sbuf_tensor("pos8", [16, 8], F32).ap()

            a2a_in = dpool.tile([B, PAY], BF16, tag="a2a_in")
            a2a_out = dpool.tile([B, PAY], BF16, tag="a2a_out")

            def cmpx(o, a, b, op):
                nc.vector.tensor_tensor(o, a, b, op)

            # ---- load q^T and penalty ----
            for hc in range(HC):
                nc.sync.dma_start(qt_sb[:, hc * 128:(hc + 1) * 128],
                                  qT[hc * 128:(hc + 1) * 128, :])
            nc.sync.dma_start(T[:], pen[:])

            # top8 candidates per 512-col block (pre-mask, f32 from PSUM)
            t8c = nc.alloc_sbuf_tensor("t8c", [128, 8 * JC], F32).ap()

            # ---- matmul S = q @ F^T ----
            for jc in range(JC):
                ftiles = []
                for hc in range(HC):
                    ft = fpool.tile([128, 512], BF16, tag="ft")
                    nc.sync.dma_start(
                        ft[:], fqt[hc * 128:(hc + 1) * 128,
                                   jc * 512:(jc + 1) * 512])
                    ftiles.append(ft)
                ps = ppool.tile([128, 512], F32, tag="ps")
                for hc in range(HC):
                    nc.tensor.matmul(ps[:], qt_sb[:, hc * 128:(hc + 1) * 128],
                                     ftiles[hc][:], start=(hc == 0),
                                     stop=(hc == HC - 1))
                sl = slice(jc * 512, (jc + 1) * 512)
                # unmasked f32 copy for exact-ish top8, on the idle ScalarE
                nc.scalar.activation(U32[:], ps[:],
                                     mybir.ActivationFunctionType.Copy)
                nc.vector.max(t8c[:, jc * 8:(jc + 1) * 8], U32[:])
                # masked bf16 chunk for the sort: S = psum + pen
                cmpx(S[:, sl], ps[:], T[:, sl], ADD)

            # ---- top8 per row = max over per-block candidates ----
            nc.vector.max(top8[:], t8c[:])

            # ---- per-row descending bitonic sort of the chunk ----
            cur, oth = S, T

            def halving(s, c0=0, c1=None):
                nonlocal cur, oth
                c1 = KC if c1 is None else c1
                a = cur[:, c0:c1].rearrange("p (b two s) -> p b two s",
                                            two=2, s=s)
                o = oth[:, c0:c1].rearrange("p (b two s) -> p b two s",
                                            two=2, s=s)
                cmpx(o[:, :, 0, :], a[:, :, 0, :], a[:, :, 1, :], MAX)
                cmpx(o[:, :, 1, :], a[:, :, 0, :], a[:, :, 1, :], MIN)
                cur, oth = oth, cur

            def sort_rounds(kmax, c0=0, c1=None):
                nonlocal cur, oth
                c1 = KC if c1 is None else c1
                for k in range(1, kmax + 1):
                    m = 1 << k
                    a = cur[:, c0:c1].rearrange("p (b m) -> p b m", m=m)
                    o = oth[:, c0:c1].rearrange("p (b m) -> p b m", m=m)
                    lo = a[:, :, 0:m // 2]
                    hi = a[:, :, m // 2:m]
                    cmpx(o[:, :, 0:m // 2], lo, hi[:, :, ::-1], MAX)
                    cmpx(o[:, :, m // 2:m], hi, lo[:, :, ::-1], MIN)
                    cur, oth = oth, cur
                    s = m // 4
                    while s >= 1:
                        halving(s, c0, c1)
                        s //= 2

            if SPLIT_PHASE4:
                kh = _log2i(KC) - 1
                sort_rounds(kh, 0, KC // 2)       # 78 stages (even): ends in S
                sort_rounds(kh, KC // 2, KC)      # 78 stages (even): ends in S
                # final round: mirror over full KC + halvings
                m = KC
                a = cur.rearrange("p (b m) -> p b m", m=m)
                o = oth.rearrange("p (b m) -> p b m", m=m)
                lo = a[:, :, 0:m // 2]
                hi = a[:, :, m // 2:m]
                cmpx(o[:, :, 0:m // 2], lo, hi[:, :, ::-1], MAX)
                cmpx(o[:, :, m // 2:m], hi, lo[:, :, ::-1], MIN)
                cur, oth = oth, cur
                s = m // 4
                while s >= 1:
                    halving(s)
                    s //= 2
            else:
                sort_rounds(_log2i(KC))

            # ---- stage for A2A ----
            nc.gpsimd.dma_start(a2a_in[:, 0:KC], cur[:])
            nc.gpsimd.dma_start(a2a_in[:, KC:PAY], top8.bitcast(BF16))

            # ---- AllToAll (16-row shards) ----
            nc.gpsimd.collective_compute(
                "AllToAll", mybir.AluOpType.bypass,
                replica_groups=[list(range(NCORES))],
                ins=[a2a_in.opt()], outs=[a2a_out.opt()])

            # ---- load merge tile + top8 gather ----
            pos = {}
            dmae = [nc.gpsimd, nc.sync, nc.scalar]
            for c in range(NCORES):
                g = (c // 2) if c % 2 == 0 else 4 + c // 2
                pos[c] = g
                dmae[c % 3].dma_start(cur[g * 16:(g + 1) * 16, :],
                                      a2a_out[c * 16:(c + 1) * 16, 0:KC])
                nc.sync.dma_start(T8[:, c * 16:(c + 1) * 16],
                                  a2a_out[c * 16:(c + 1) * 16, KC:PAY])

            nc.vector.max(pos8[:], T8.bitcast(F32))

            # ---- merge 8 sorted runs ----
            def cross(pairs, rev, skip_v=False):
                nonlocal cur, oth
                if not skip_v:
                    for i, (lc, uc) in enumerate(pairs):
                        dmae[i % 3].dma_start(
                            V64[i * 16:(i + 1) * 16, :],
                            cur[pos[lc] * 16:(pos[lc] + 1) * 16, :])
                    vin = V64
                else:
                    vin = cur[0:64, :]
                for i, (lc, uc) in enumerate(pairs):
                    dmae[(i + 1) % 3].dma_start(
                        Y64[i * 16:(i + 1) * 16, :],
                        cur[pos[uc] * 16:(pos[uc] + 1) * 16, :])
                y = Y64[:, ::-1] if rev else Y64[:]
                v = vin[:, ::-1] if rev else vin[:]
                cmpx(oth[0:64, :], vin[:], y, MAX)
                cmpx(oth[64:128, :], Y64[:], v, MIN)
                for i, (lc, uc) in enumerate(pairs):
                    pos[lc] = i
                    pos[uc] = 4 + i
                cur, oth = oth, cur

            def free_stages():
                s = KC // 2
                while s >= 1:
                    halving(s)
                    s //= 2

            cross([(0, 1), (2, 3), (4, 5), (6, 7)], rev=True, skip_v=True)
            free_stages()
            cross([(0, 3), (1, 2), (4, 7), (5, 6)], rev=True)
            cross([(0, 1), (2, 3), (4, 5), (6, 7)], rev=False)
            free_stages()
            cross([(0, 7), (1, 6), (2, 5), (3, 4)], rev=True)
            cross([(0, 2), (1, 3), (4, 6), (5, 7)], rev=False)
            cross([(0, 1), (2, 3), (4, 5), (6, 7)], rev=False)
            free_stages()

            # ---- convert to f32 and write outputs ----
            grp_chunk = sorted(range(8), key=lambda c: pos[c])
            fin = cur
            nc.scalar.activation(OUT32[:], fin[:],
                                 mybir.ActivationFunctionType.Copy)
            R2 = out.ap().flatten().rearrange("(r x) -> r x", x=TOPK * OUTC)
            for t in range(TOPK):
                for g in range(8):
                    cg = grp_chunk[g]
                    L = KC if cg < 7 else LAST
                    dst = R2[:, t * OUTC + 1 + cg * KC:
                             t * OUTC + 1 + cg * KC + L]
                    src = OUT32[g * 16:(g + 1) * 16, 0:L]
                    eng = dmae[(t * 8 + g) % 3]
                    eng.dma_start(dst, src)
                with nc.allow_non_contiguous_dma(reason="16 scattered f32"):
                    nc.sync.dma_start(R2[:, t * OUTC:t * OUTC + 1],
                                      pos8[:, t:t + 1])

    nc.compile()
    return nc


_NC_CACHE = {}


def _get_nc():
    if "nc" not in _NC_CACHE:
        _NC_CACHE["nc"] = build_nc()
    return _NC_CACHE["nc"]


def host_inputs(liner_q, feature_queue, label_q, label_queue, KC=KC,
                T_temp=T_TEMP):
    import jax.numpy as jnp

    lq = np.asarray(liner_q, dtype=np.float32)
    fq = np.asarray(feature_queue, dtype=np.float32)
    lbq = np.asarray(label_q).reshape(-1)
    lbQ = np.asarray(label_queue).reshape(-1)
    nrm = np.sqrt((lq * lq).sum(axis=1, keepdims=True))
    q = (lq / nrm / np.float32(T_temp)).astype(np.float32)
    qT = np.asarray(jnp.asarray(q.T, dtype=jnp.bfloat16))
    in_maps = []
    for c in range(NCORES):
        sl = slice(c * KC, (c + 1) * KC)
        fqt_c = np.asarray(jnp.asarray(fq[sl, :].T, dtype=jnp.bfloat16))
        pen_c = np.asarray(jnp.asarray(
            np.where(lbq[:, None] == lbQ[None, sl], np.float32(-1e38),
                     np.float32(0.0)), dtype=jnp.bfloat16))
        in_maps.append({"qT": np.ascontiguousarray(qT),
                        "fqt": np.ascontiguousarray(fqt_c),
                        "pen": np.ascontiguousarray(pen_c)})
    return in_maps


def _get_runner():
    """Cached jitted SPMD executable (avoids re-trace/re-compile per call)."""
    if "runner" in _NC_CACHE:
        return _NC_CACHE["runner"]
    import jax
    from jax.sharding import Mesh, NamedSharding, PartitionSpec
    from jax.experimental.shard_map import shard_map
    from concourse import bass2jax

    nc = _get_nc()
    partition_name = (nc.partition_id_tensor.name
                      if nc.partition_id_tensor else None)
    in_names, out_names, out_avals, out_shapes = [], [], [], []
    for alloc in nc.m.functions[0].allocations:
        if not isinstance(alloc, mybir.MemoryLocationSet):
            continue
        name = alloc.memorylocations[0].name
        if alloc.kind == "ExternalInput":
            if name != partition_name:
                in_names.append(name)
        elif alloc.kind == "ExternalOutput":
            out_names.append(name)
            shape = tuple(alloc.tensor_shape)
            dtype = mybir.dt.np(alloc.dtype)
            out_avals.append(jax.core.ShapedArray(shape, dtype))
            out_shapes.append((shape, dtype))
    n_params = len(in_names)
    all_in = list(in_names) + list(out_names)
    if partition_name is not None:
        all_in.append(partition_name)

    def _body(*args):
        operands = list(args)
        if partition_name is not None:
            operands.append(bass2jax.partition_id_tensor())
        return tuple(bass2jax._bass_exec_p.bind(
            *operands, out_avals=tuple(out_avals), in_names=tuple(all_in),
            out_names=tuple(out_names), lowering_input_output_aliases=(),
            sim_require_finite=True, sim_require_nnan=True, nc=nc))

    devices = jax.devices()[:NCORES]
    mesh = Mesh(np.asarray(devices), ("core",))
    fn = jax.jit(
        shard_map(_body, mesh=mesh,
                  in_specs=(PartitionSpec("core"),) * (n_params + len(out_names)),
                  out_specs=(PartitionSpec("core"),) * len(out_names),
                  check_rep=False),
        keep_unused=True)
    sharding = NamedSharding(mesh, PartitionSpec("core"))

    import jax.numpy as jnp
    _zeros = jax.jit(
        lambda: tuple(jnp.zeros((NCORES * s[0], *s[1:]), d)
                      for (s, d) in out_shapes),
        out_shardings=tuple(sharding for _ in out_shapes))

    def prepare(in_maps):
        per_core = [[np.asarray(m[nm]) for nm in in_names] for m in in_maps]
        concat_in = [np.concatenate([per_core[c][i] for c in range(NCORES)],
                                    axis=0) for i in range(n_params)]
        dev_in = [jax.device_put(a, sharding) for a in concat_in]
        # outputs are not donated; reuse one zeros buffer across calls
        dev_zeros = _zeros()
        return dev_in, dev_zeros

    def execute(prepared):
        dev_in, dev_zeros = prepared
        return fn(*dev_in, *dev_zeros)

    def runner(in_maps):
        outs = execute(prepare(in_maps))
        return np.asarray(outs[0])  # [NCORES*80, OUTC], core-major

    runner.prepare = prepare
    runner.execute = execute
    _NC_CACHE["runner"] = runner
    return runner


def run(inputs, trace=False, **kw):
    """Reference-path runner (used by test.py; returns BassKernelResults)."""
    nc = _get_nc()
    in_maps = host_inputs(**inputs)
    res = run_bass_kernel_spmd(nc, in_maps, core_ids=list(range(NCORES)),
                               trace=trace, **kw)
    full = np.concatenate([r["out"] for r in res.results], axis=0)
    return full, res


def kernel(liner_q, feature_queue, label_q, label_queue):
    inputs = dict(liner_q=liner_q, feature_queue=feature_queue,
                  label_q=label_q, label_queue=label_queue)
    try:
        runner = _get_runner()
        return runner(host_inputs(**inputs))
    except Exception:
        full, _ = run(inputs)
        return full


# revision 14
# speedup vs baseline: 4.6945x; 1.0949x over previous
"""Trainium2 Bass kernel for nn_BERT_KNNCL_35527969473209 (retrieval_knn).

Contract: kernel(**inputs) takes the FULL inputs (liner_q [128,768] f32,
feature_queue [65536,768] f32, label_q [128] int, label_queue [65536] int)
and returns the FULL output [640, 64513] f32, matching:

    q = l2norm(liner_q); cos = q @ feature_queue.T
    pos = top_k(cos, 5) -> [640,1]
    neg = sort_desc(where(label match, -inf, cos))[:, :64512], rows repeated 5x
    out = concat([pos, neg], -1) / 0.07

Strategy (SPMD over 8 NeuronCores, queue-dim sharded), v2 = bf16 sort:
  host: l2norm+1/T fold into q^T (bf16); per-core feature chunk transposed
        (bf16); per-core penalty matrix (-1e38 at label matches, bf16).
  core c: S = q^T.T @ fqt_c  [128 x 8192] (PE bf16 -> PSUM f32)
          top8/row via DVE InstMax (f32, pre-mask); S = PSUM + pen (bf16);
          per-row descending bitonic sort of the 8192-chunk in bf16 using
          DVE scalar_tensor_tensor (bypass,max/min) - the InstTensorScalarPtr
          form runs at 4x (2-byte packed SBUF operands) vs 1x for the
          InstTensorTensor form at fp32;
          AllToAll 16-row shards (bf16 payload + bitcast f32 top8);
          bitonic merge of 8 sorted runs (DVE stt + DMA relabel staged
          across 3 queues); top5 = max8 of gathered top8s (f32);
          convert merged rows to f32 on the Scalar engine;
          write the [80 x 64513] shard (5x row replication via DMA).
  host: concatenate the 8 shards.

Numerics: bf16 quantization of the (already 1/T-scaled) sims bounds the
elementwise error by ~0.5 ULP(bf16) ~ 0.25 at |x|~82 -> rel-to-scale
~3e-3, well inside the 2e-2 gate; the pos column is computed from f32
top8 candidates. Sorting quantized keys is exact on the quantized values,
so no error accumulates across the 136 compare-exchange stages.
"""

import sys

import numpy as np

for _p in ("/opt/trn_rl_repo", "/root/.axon_site/_ro/trn_rl_repo"):
    if _p not in sys.path:
        sys.path.append(_p)

import concourse.bass as bass  # noqa: E402
import concourse.tile as tile  # noqa: E402
from concourse import bacc, mybir  # noqa: E402
from concourse.bass_utils import run_bass_kernel_spmd  # noqa: E402

F32 = mybir.dt.float32
BF16 = mybir.dt.bfloat16
MAX = mybir.AluOpType.max
MIN = mybir.AluOpType.min
ADD = mybir.AluOpType.add
BYP = mybir.AluOpType.bypass

NCORES = 8
B = 128
NROW = B // NCORES
NLBL = 64
TOPK = 5
KC = 8192
H = 768
T_TEMP = 0.07
# Emit phase-4 rounds 1..log2(KC)-1 separately per column half so the left
# half's sort can overlap the right half's matmul/DMA stream.
SPLIT_PHASE4 = True


def _log2i(n):
    k = n.bit_length() - 1
    assert (1 << k) == n
    return k


def build_nc(KC=KC, H=H):
    K = KC * NCORES
    POS = K // NLBL
    LAST = KC - POS
    NEG = K - POS
    OUTC = NEG + 1
    HC = H // 128
    JC = KC // 512
    PAY = KC + 16  # bf16 payload cols: sorted chunk + bitcast f32 top8

    nc = bacc.Bacc("TRN2", target_bir_lowering=False, debug=False,
                   num_devices=NCORES)

    qT = nc.dram_tensor("qT", [H, B], BF16, kind="ExternalInput")
    fqt = nc.dram_tensor("fqt", [H, KC], BF16, kind="ExternalInput")
    pen = nc.dram_tensor("pen", [B, KC], BF16, kind="ExternalInput")
    out = nc.dram_tensor("out", [NROW * TOPK, OUTC], F32, kind="ExternalOutput")

    with tile.TileContext(nc) as tc:
        with (
            tc.tile_pool(name="fq", bufs=12) as fpool,
            tc.tile_pool(name="psum", bufs=4, space="PSUM") as ppool,
            tc.tile_pool(name="dram", bufs=1, space="DRAM") as dpool,
        ):
            S = nc.alloc_sbuf_tensor("S", [128, KC], BF16).ap()
            T = nc.alloc_sbuf_tensor("T", [128, KC], BF16).ap()
            V64 = nc.alloc_sbuf_tensor("V64", [64, KC], BF16).ap()
            Y64 = nc.alloc_sbuf_tensor("Y64", [64, KC], BF16).ap()
            OUT32 = nc.alloc_sbuf_tensor("OUT32", [128, KC], F32).ap()
            qt_sb = nc.alloc_sbuf_tensor("qt_sb", [128, H], BF16).ap()
            U32 = nc.alloc_sbuf_tensor("U32", [128, 512], F32).ap()
            top8 = nc.alloc_sbuf_tensor("top8", [128, 8], F32).ap()
            T8 = nc.alloc_sbuf_tensor("T8", [16, 128], BF16).ap()
            pos8 = nc.alloc_sbuf_tensor("pos8", [16, 8], F32).ap()

            a2a_in = dpool.tile([B, PAY], BF16, tag="a2a_in")
            a2a_out = dpool.tile([B, PAY], BF16, tag="a2a_out")

            def cmpx(o, a, b, op):
                nc.vector.tensor_tensor(o, a, b, op)

            # Compare-exchange split across DVE (bf16 packed = 2x) and Pool
            # (1x but concurrent). frac = DVE's share of the columns.
            def cmpx2(o, a, b, op, nsplit, frac):
                """Emit op over [..., nsplit, ...] APs split along the dim of
                size nsplit (must be dim 1 of each AP) between DVE and Pool."""
                nd = max(1, min(nsplit - 1, int(round(nsplit * frac))))
                nc.vector.tensor_tensor(o[:, :nd], a[:, :nd], b[:, :nd], op)
                nc.gpsimd.tensor_tensor(o[:, nd:], a[:, nd:], b[:, nd:], op)

            # ---- load q^T and penalty ----
            for hc in range(HC):
                nc.sync.dma_start(qt_sb[:, hc * 128:(hc + 1) * 128],
                                  qT[hc * 128:(hc + 1) * 128, :])
            nc.sync.dma_start(T[:], pen[:])

            # top8 candidates per 512-col block (pre-mask, f32 from PSUM)
            t8c = nc.alloc_sbuf_tensor("t8c", [128, 8 * JC], F32).ap()

            # ---- matmul S = q @ F^T ----
            for jc in range(JC):
                ftiles = []
                for hc in range(HC):
                    ft = fpool.tile([128, 512], BF16, tag="ft")
                    nc.sync.dma_start(
                        ft[:], fqt[hc * 128:(hc + 1) * 128,
                                   jc * 512:(jc + 1) * 512])
                    ftiles.append(ft)
                ps = ppool.tile([128, 512], F32, tag="ps")
                for hc in range(HC):
                    nc.tensor.matmul(ps[:], qt_sb[:, hc * 128:(hc + 1) * 128],
                                     ftiles[hc][:], start=(hc == 0),
                                     stop=(hc == HC - 1))
                sl = slice(jc * 512, (jc + 1) * 512)
                # unmasked f32 copy for exact-ish top8, on the idle ScalarE
                nc.scalar.activation(U32[:], ps[:],
                                     mybir.ActivationFunctionType.Copy)
                nc.vector.max(t8c[:, jc * 8:(jc + 1) * 8], U32[:])
                # masked bf16 chunk for the sort: S = psum + pen
                cmpx(S[:, sl], ps[:], T[:, sl], ADD)

            # ---- top8 per row = max over per-block candidates ----
            nc.vector.max(top8[:], t8c[:])

            # ---- per-row descending bitonic sort of the chunk ----
            cur, oth = S, T

            def halving(s, c0=0, c1=None):
                nonlocal cur, oth
                c1 = KC if c1 is None else c1
                a = cur[:, c0:c1].rearrange("p (b two s) -> p b two s",
                                            two=2, s=s)
                o = oth[:, c0:c1].rearrange("p (b two s) -> p b two s",
                                            two=2, s=s)
                cmpx(o[:, :, 0, :], a[:, :, 0, :], a[:, :, 1, :], MAX)
                cmpx(o[:, :, 1, :], a[:, :, 0, :], a[:, :, 1, :], MIN)
                cur, oth = oth, cur

            def mirror(m, c0=0, c1=None):
                nonlocal cur, oth
                c1 = KC if c1 is None else c1
                a = cur[:, c0:c1].rearrange("p (b m) -> p b m", m=m)
                o = oth[:, c0:c1].rearrange("p (b m) -> p b m", m=m)
                lo = a[:, :, 0:m // 2]
                hi = a[:, :, m // 2:m]
                cmpx(o[:, :, 0:m // 2], lo, hi[:, :, ::-1], MAX)
                cmpx(o[:, :, m // 2:m], hi, lo[:, :, ::-1], MIN)
                cur, oth = oth, cur

            def sort_rounds(kmax, c0=0, c1=None):
                c1 = KC if c1 is None else c1
                for k in range(1, kmax + 1):
                    m = 1 << k
                    mirror(m, c0, c1)
                    s = m // 4
                    while s >= 1:
                        halving(s, c0, c1)
                        s //= 2

            if SPLIT_PHASE4:
                kh = _log2i(KC) - 1
                sort_rounds(kh, 0, KC // 2)       # 78 stages (even): ends in S
                sort_rounds(kh, KC // 2, KC)      # 78 stages (even): ends in S
                # final round: mirror over full KC + halvings
                mirror(KC)
                s = KC // 4
                while s >= 1:
                    halving(s)
                    s //= 2
            else:
                sort_rounds(_log2i(KC))

            # ---- stage for A2A ----
            nc.gpsimd.dma_start(a2a_in[:, 0:KC], cur[:])
            nc.gpsimd.dma_start(a2a_in[:, KC:PAY], top8.bitcast(BF16))

            # ---- AllToAll (16-row shards) ----
            nc.gpsimd.collective_compute(
                "AllToAll", mybir.AluOpType.bypass,
                replica_groups=[list(range(NCORES))],
                ins=[a2a_in.opt()], outs=[a2a_out.opt()])

            # ---- load merge tile + top8 gather ----
            pos = {}
            dmae = [nc.gpsimd, nc.sync, nc.scalar]
            for c in range(NCORES):
                g = (c // 2) if c % 2 == 0 else 4 + c // 2
                pos[c] = g
                dmae[c % 3].dma_start(cur[g * 16:(g + 1) * 16, :],
                                      a2a_out[c * 16:(c + 1) * 16, 0:KC])
                nc.sync.dma_start(T8[:, c * 16:(c + 1) * 16],
                                  a2a_out[c * 16:(c + 1) * 16, KC:PAY])

            nc.vector.max(pos8[:], T8.bitcast(F32))

            # ---- merge 8 sorted runs ----
            def cross(pairs, rev, skip_v=False):
                nonlocal cur, oth
                cpe = [nc.sync, nc.scalar]
                if not skip_v:
                    for i, (lc, uc) in enumerate(pairs):
                        cpe[i % 2].dma_start(
                            V64[i * 16:(i + 1) * 16, :],
                            cur[pos[lc] * 16:(pos[lc] + 1) * 16, :])
                    vin = V64
                else:
                    vin = cur[0:64, :]
                for i, (lc, uc) in enumerate(pairs):
                    cpe[(i + 1) % 2].dma_start(
                        Y64[i * 16:(i + 1) * 16, :],
                        cur[pos[uc] * 16:(pos[uc] + 1) * 16, :])
                y = Y64[:, ::-1] if rev else Y64[:]
                v = vin[:, ::-1] if rev else vin[:]
                cmpx(oth[0:64, :], vin[:], y, MAX)
                cmpx(oth[64:128, :], Y64[:], v, MIN)
                for i, (lc, uc) in enumerate(pairs):
                    pos[lc] = i
                    pos[uc] = 4 + i
                cur, oth = oth, cur

            def free_stages():
                s = KC // 2
                while s >= 1:
                    halving(s)
                    s //= 2

            cross([(0, 1), (2, 3), (4, 5), (6, 7)], rev=True, skip_v=True)
            free_stages()
            cross([(0, 3), (1, 2), (4, 7), (5, 6)], rev=True)
            cross([(0, 1), (2, 3), (4, 5), (6, 7)], rev=False)
            free_stages()
            cross([(0, 7), (1, 6), (2, 5), (3, 4)], rev=True)
            cross([(0, 2), (1, 3), (4, 6), (5, 7)], rev=False)
            cross([(0, 1), (2, 3), (4, 5), (6, 7)], rev=False)
            # final fixup round, column-half pipelined with the output path:
            # s=4096 couples the halves; s<=2048 are independent per half, so
            # the left half's convert+DMA overlaps the right half's stages.
            halving(KC // 2)
            grp_chunk = sorted(range(8), key=lambda c: pos[c])
            R2 = out.ap().flatten().rearrange("(r x) -> r x", x=TOPK * OUTC)
            dmo = [nc.gpsimd, nc.sync, nc.scalar]
            for half, (c0, c1) in enumerate(((0, KC // 2), (KC // 2, KC))):
                s = KC // 4
                while s >= 1:
                    halving(s, c0, c1)
                    s //= 2
                nc.scalar.activation(OUT32[:, c0:c1], cur[:, c0:c1],
                                     mybir.ActivationFunctionType.Copy)
                nq = 3
                for t in range(TOPK):
                    for g in range(8):
                        cg = grp_chunk[g]
                        L = KC if cg < 7 else LAST
                        lo, hi = c0, min(c1, L)
                        if hi <= lo:
                            continue
                        dst = R2[:, t * OUTC + 1 + cg * KC + lo:
                                 t * OUTC + 1 + cg * KC + hi]
                        src = OUT32[g * 16:(g + 1) * 16, lo:hi]
                        dmo[(t * 8 + g) % nq].dma_start(dst, src)
            for t in range(TOPK):
                with nc.allow_non_contiguous_dma(reason="16 scattered f32"):
                    nc.sync.dma_start(R2[:, t * OUTC:t * OUTC + 1],
                                      pos8[:, t:t + 1])

    nc.compile()
    return nc


_NC_CACHE = {}


def _get_nc():
    if "nc" not in _NC_CACHE:
        _NC_CACHE["nc"] = build_nc()
    return _NC_CACHE["nc"]


def host_inputs(liner_q, feature_queue, label_q, label_queue, KC=KC,
                T_temp=T_TEMP):
    import jax.numpy as jnp

    lq = np.asarray(liner_q, dtype=np.float32)
    fq = np.asarray(feature_queue, dtype=np.float32)
    lbq = np.asarray(label_q).reshape(-1)
    lbQ = np.asarray(label_queue).reshape(-1)
    nrm = np.sqrt((lq * lq).sum(axis=1, keepdims=True))
    q = (lq / nrm / np.float32(T_temp)).astype(np.float32)
    qT = np.asarray(jnp.asarray(q.T, dtype=jnp.bfloat16))
    in_maps = []
    for c in range(NCORES):
        sl = slice(c * KC, (c + 1) * KC)
        fqt_c = np.asarray(jnp.asarray(fq[sl, :].T, dtype=jnp.bfloat16))
        pen_c = np.asarray(jnp.asarray(
            np.where(lbq[:, None] == lbQ[None, sl], np.float32(-1e38),
                     np.float32(0.0)), dtype=jnp.bfloat16))
        in_maps.append({"qT": np.ascontiguousarray(qT),
                        "fqt": np.ascontiguousarray(fqt_c),
                        "pen": np.ascontiguousarray(pen_c)})
    return in_maps


def _get_runner():
    """Cached jitted SPMD executable (avoids re-trace/re-compile per call)."""
    if "runner" in _NC_CACHE:
        return _NC_CACHE["runner"]
    import jax
    from jax.sharding import Mesh, NamedSharding, PartitionSpec
    from jax.experimental.shard_map import shard_map
    from concourse import bass2jax

    nc = _get_nc()
    partition_name = (nc.partition_id_tensor.name
                      if nc.partition_id_tensor else None)
    in_names, out_names, out_avals, out_shapes = [], [], [], []
    for alloc in nc.m.functions[0].allocations:
        if not isinstance(alloc, mybir.MemoryLocationSet):
            continue
        name = alloc.memorylocations[0].name
        if alloc.kind == "ExternalInput":
            if name != partition_name:
                in_names.append(name)
        elif alloc.kind == "ExternalOutput":
            out_names.append(name)
            shape = tuple(alloc.tensor_shape)
            dtype = mybir.dt.np(alloc.dtype)
            out_avals.append(jax.core.ShapedArray(shape, dtype))
            out_shapes.append((shape, dtype))
    n_params = len(in_names)
    all_in = list(in_names) + list(out_names)
    if partition_name is not None:
        all_in.append(partition_name)

    def _body(*args):
        operands = list(args)
        if partition_name is not None:
            operands.append(bass2jax.partition_id_tensor())
        return tuple(bass2jax._bass_exec_p.bind(
            *operands, out_avals=tuple(out_avals), in_names=tuple(all_in),
            out_names=tuple(out_names), lowering_input_output_aliases=(),
            sim_require_finite=True, sim_require_nnan=True, nc=nc))

    devices = jax.devices()[:NCORES]
    mesh = Mesh(np.asarray(devices), ("core",))
    fn = jax.jit(
        shard_map(_body, mesh=mesh,
                  in_specs=(PartitionSpec("core"),) * (n_params + len(out_names)),
                  out_specs=(PartitionSpec("core"),) * len(out_names),
                  check_rep=False),
        keep_unused=True)
    sharding = NamedSharding(mesh, PartitionSpec("core"))

    import jax.numpy as jnp
    _zeros = jax.jit(
        lambda: tuple(jnp.zeros((NCORES * s[0], *s[1:]), d)
                      for (s, d) in out_shapes),
        out_shardings=tuple(sharding for _ in out_shapes))

    def prepare(in_maps):
        per_core = [[np.asarray(m[nm]) for nm in in_names] for m in in_maps]
        concat_in = [np.concatenate([per_core[c][i] for c in range(NCORES)],
                                    axis=0) for i in range(n_params)]
        dev_in = [jax.device_put(a, sharding) for a in concat_in]
        # outputs are not donated; reuse one zeros buffer across calls
        dev_zeros = _zeros()
        return dev_in, dev_zeros

    def execute(prepared):
        dev_in, dev_zeros = prepared
        return fn(*dev_in, *dev_zeros)

    def runner(in_maps):
        outs = execute(prepare(in_maps))
        return np.asarray(outs[0])  # [NCORES*80, OUTC], core-major

    runner.prepare = prepare
    runner.execute = execute
    _NC_CACHE["runner"] = runner
    return runner


def run(inputs, trace=False, **kw):
    """Reference-path runner (used by test.py; returns BassKernelResults)."""
    nc = _get_nc()
    in_maps = host_inputs(**inputs)
    res = run_bass_kernel_spmd(nc, in_maps, core_ids=list(range(NCORES)),
                               trace=trace, **kw)
    full = np.concatenate([r["out"] for r in res.results], axis=0)
    return full, res


def kernel(liner_q, feature_queue, label_q, label_queue):
    inputs = dict(liner_q=liner_q, feature_queue=feature_queue,
                  label_q=label_q, label_queue=label_queue)
    try:
        runner = _get_runner()
        return runner(host_inputs(**inputs))
    except Exception:
        full, _ = run(inputs)
        return full
